# revision 18
# baseline (speedup 1.0000x reference)
"""CGMM layer (segment_reduce) Trainium2 kernel.

Math: every per-node quantity depends on the node only through its discrete
label x_n (64 values), so the kernel reduces to:
  1. a tiny 64x24 table build from B/Pi on device (softmaxes, posterior,
     max/argmax over C via DVE 32x32 block transposes, per-label likelihood),
  2. a table gather for all N nodes done on the tensor engine: one-hot(x)
     built by DVE is_equal against a per-partition label column (x is fed
     pre-replicated across 32 partitions, 4 node-slots per column), then
     block-diagonal matmuls; fp32 table precision is kept with a bf16 hi/lo
     weight split accumulated in PSUM,
  3. an unsegmented prefix sum (DVE tensor_tensor_scan) of the per-node
     likelihood; the host samples it at graph boundaries (known from the
     sorted `batch`) and differences to get per-graph segment sums.

Sharding: data-parallel over nodes, 37500 nodes per core on 8 cores; B/Pi
replicated; outputs disjoint per core except graph sums, stitched on host.

Output column layout of the gather matmul (96 rows): col = 4*j + s for
output j (0..7 h_vals, 8..15 h_idx, 16..23 lik) and node-slot s (0..3),
so h_vals rows = [0:32), h_idx = [32:64), lik = [64:96) - contiguous blocks.
"""

import numpy as np
import ml_dtypes
from contextlib import ExitStack

import concourse.bass as bass
import concourse.tile as tile
import concourse.mybir as mybir
from concourse import bacc
from concourse.bass_utils import run_bass_kernel_spmd

N = 300000
C = 32
M = 64
NGEN = 8
G = 8192
NCORES = 8
NPC = N // NCORES          # 37500 nodes per core
SLOT = 9376                # nodes per slot; 4 slots/core, last 4 nodes pad
NSLOT = 4
BIG = 10000.0
NMM = 512                  # matmul free-dim chunk

_cache = {}


def _build_nc(reps: int = 1):
    nc = bacc.Bacc("TRN2", target_bir_lowering=False, debug=False,
                   num_devices=NCORES)
    f32, bf16 = mybir.dt.float32, mybir.dt.bfloat16

    Bp = nc.declare_dram_parameter("Bp", [C, M * NGEN], f32, isOutput=False)
    Pip = nc.declare_dram_parameter("Pip", [C, NGEN], f32, isOutput=False)
    xb_in = nc.declare_dram_parameter("xb", [128, SLOT], bf16, isOutput=False)
    ilo_in = nc.declare_dram_parameter("ilo", [128, 1], f32, isOutput=False)
    ihi_in = nc.declare_dram_parameter("ihi", [128, 1], f32, isOutput=False)
    iotac_in = nc.declare_dram_parameter("iotac", [C, M * NGEN], f32, isOutput=False)

    out_ext = nc.declare_dram_parameter("out", [128, SLOT], f32, isOutput=True)

    with tile.TileContext(nc, num_cores=NCORES) as tc:
        with ExitStack() as ctx:
            small = ctx.enter_context(tc.tile_pool(name="small", bufs=1))
            big = ctx.enter_context(tc.tile_pool(name="big", bufs=1))
            psum = ctx.enter_context(tc.tile_pool(name="ps", bufs=6, space="PSUM"))
            dpool = ctx.enter_context(tc.tile_pool(name="dscratch", bufs=1, space="DRAM"))
            dW = dpool.tile([M, 24], f32)

            import contextlib
            loop_ctx = tc.For_i(0, reps, 1) if reps > 1 else contextlib.nullcontext()
            with loop_ctx:
                _emit_body(nc, tc, locals())

    nc.finalize()
    return nc


def _emit_body(nc, tc, env):
    f32, bf16 = mybir.dt.float32, mybir.dt.bfloat16
    small, big, psum = env["small"], env["big"], env["psum"]
    dW = env["dW"]
    Bp, Pip, xb_in = env["Bp"], env["Pip"], env["xb_in"]
    ilo_in, ihi_in, iotac_in = env["ilo_in"], env["ihi_in"], env["iotac_in"]
    out_ext = env["out_ext"]
    if True:
        if True:
            # ---------------- inputs ----------------
            xb = big.tile([128, SLOT], bf16)
            nc.sync.dma_start(xb[:], xb_in[:])
            ilo = small.tile([128, 1], f32)
            nc.gpsimd.dma_start(ilo[:], ilo_in[:])
            ihi = small.tile([128, 1], f32)
            nc.gpsimd.dma_start(ihi[:], ihi_in[:])
            iotac = small.tile([C, 512], f32)
            nc.gpsimd.dma_start(iotac[:], iotac_in[:])
            Bt = small.tile([C, 512], f32)
            nc.scalar.dma_start(Bt[:], Bp[:])
            Pit = small.tile([C, NGEN], f32)
            nc.scalar.dma_start(Pit[:], Pip[:])

            # ---------------- table build (tiny) ----------------
            warm = small.tile([1, 8], f32)
            nc.vector.memset(warm[:], 0.0)
            nc.scalar.activation(warm[:], warm[:],
                                 mybir.ActivationFunctionType.Exp)
            tc.cur_priority -= 1000
            eB = small.tile([C, 512], f32)
            nc.scalar.activation(eB[:], Bt[:], mybir.ActivationFunctionType.Exp)
            sB = small.tile([C, NGEN], f32)
            nc.vector.tensor_reduce(
                sB[:], eB[:].rearrange("p (g m) -> p g m", m=M),
                axis=mybir.AxisListType.X, op=mybir.AluOpType.add)
            rB = small.tile([C, NGEN], f32)
            nc.vector.reciprocal(rB[:], sB[:])

            ePi = small.tile([C, NGEN], f32)
            nc.scalar.activation(ePi[:], Pit[:], mybir.ActivationFunctionType.Exp)
            piP = small.tile([C, 32], f32)
            nc.vector.memset(piP[:], 0.0)
            nc.vector.tensor_copy(piP[:, 0:NGEN], ePi[:])
            piT = small.tile([C, 32], f32)
            nc.vector.transpose(piT[:], piP[:])          # piT[g, c]
            sPi = small.tile([C, 1], f32)
            nc.vector.tensor_reduce(sPi[:], piT[:], axis=mybir.AxisListType.X,
                                    op=mybir.AluOpType.add)
            rPi = small.tile([C, 1], f32)
            nc.vector.reciprocal(rPi[:], sPi[:])
            smPiT = small.tile([C, 32], f32)
            nc.vector.tensor_scalar(smPiT[:], piT[:], rPi[:, 0:1], None,
                                    op0=mybir.AluOpType.mult)
            smPi32 = small.tile([C, 32], f32)
            nc.vector.transpose(smPi32[:], smPiT[:])     # smPi32[c, g]

            # num = eB * (rB * smPi) broadcast over m
            fPi = small.tile([C, NGEN], f32)
            nc.vector.tensor_tensor(fPi[:], rB[:], smPi32[:, 0:NGEN],
                                    op=mybir.AluOpType.mult)
            num = small.tile([C, 512], f32)
            nc.vector.tensor_tensor(
                num[:].rearrange("p (g m) -> p g m", m=M),
                eB[:].rearrange("p (g m) -> p g m", m=M),
                fPi[:].unsqueeze(2).broadcast_to((C, NGEN, M)),
                op=mybir.AluOpType.mult)

            # transpose to (mg-within-block, c) for the C-axis reductions
            numT = small.tile([C, 512], f32)
            nc.vector.transpose(numT[:], num[:])
            den = small.tile([C, 16], f32)
            nc.vector.tensor_reduce(
                den[:], numT[:].rearrange("p (k q) -> p k q", q=32),
                axis=mybir.AxisListType.X, op=mybir.AluOpType.add)
            rden = small.tile([C, 16], f32)
            nc.vector.reciprocal(rden[:], den[:])
            postT = small.tile([C, 512], f32)
            nc.vector.tensor_tensor(
                postT[:].rearrange("p (k q) -> p k q", q=32),
                numT[:].rearrange("p (k q) -> p k q", q=32),
                rden[:].unsqueeze(2).broadcast_to((C, 16, 32)),
                op=mybir.AluOpType.mult)
            lognT = small.tile([C, 512], f32)
            nc.scalar.activation(lognT[:], numT[:], mybir.ActivationFunctionType.Ln)
            plT = small.tile([C, 512], f32)
            nc.vector.tensor_tensor(plT[:], postT[:], lognT[:],
                                    op=mybir.AluOpType.mult)
            likmg = small.tile([C, 16], f32)
            nc.vector.tensor_reduce(
                likmg[:], plT[:].rearrange("p (k q) -> p k q", q=32),
                axis=mybir.AxisListType.X, op=mybir.AluOpType.add)
            hvmg = small.tile([C, 16], f32)
            nc.vector.tensor_reduce(
                hvmg[:], postT[:].rearrange("p (k q) -> p k q", q=32),
                axis=mybir.AxisListType.X, op=mybir.AluOpType.max)
            mask = small.tile([C, 512], f32)
            nc.vector.tensor_tensor(
                mask[:].rearrange("p (k q) -> p k q", q=32),
                postT[:].rearrange("p (k q) -> p k q", q=32),
                hvmg[:].unsqueeze(2).broadcast_to((C, 16, 32)),
                op=mybir.AluOpType.is_equal)
            cand2 = small.tile([C, 512], f32)
            nc.vector.scalar_tensor_tensor(cand2[:], mask[:], -BIG, iotac[:],
                                           op0=mybir.AluOpType.mult,
                                           op1=mybir.AluOpType.add)
            himg = small.tile([C, 16], f32)
            nc.vector.tensor_reduce(
                himg[:], cand2[:].rearrange("p (k q) -> p k q", q=32),
                axis=mybir.AxisListType.X, op=mybir.AluOpType.min)

            # dump tables into dW[m, 8t+g]: tile element (p=8a+b, k) is the
            # value for mg = 32k + p, i.e. m = 4k + a, g = b
            dWv = dW[:, :].rearrange("(mhi p) (t g) -> t mhi p g", mhi=2, g=8)
            dump_engs = (nc.sync, nc.scalar, nc.gpsimd)
            for t, tmg in enumerate((hvmg, himg, likmg)):
                tv = tmg[:, :].rearrange("p (g mhi) -> mhi p g", mhi=2)
                for mhi in range(2):
                    dump_engs[t].dma_start(dWv[t, mhi], tv[mhi])

            # fp32 block-diagonal weights, then bf16 hi/lo split.
            # W[32s + m, 4j + s] = T[m, j]
            Wlo = small.tile([128, 96], f32)
            nc.vector.memset(Wlo[:], 0.0)
            Whi = small.tile([128, 96], f32)
            nc.vector.memset(Whi[:], 0.0)
            for s in range(NSLOT):
                dst = slice(32 * s, 32 * s + 32)
                wv_lo = Wlo[dst, :].rearrange("m (j four) -> m four j", four=4)
                wv_hi = Whi[dst, :].rearrange("m (j four) -> m four j", four=4)
                nc.sync.dma_start(wv_lo[:, s], dW[0:32, :])
                nc.scalar.dma_start(wv_hi[:, s], dW[32:64, :])
            W1 = {}
            W2 = {}
            for name, Wf in (("lo", Wlo), ("hi", Whi)):
                w1 = small.tile([128, 96], bf16, tag=f"w1{name}")
                nc.vector.tensor_copy(w1[:], Wf[:])
                w1f = small.tile([128, 96], f32, tag=f"w1f{name}")
                nc.vector.tensor_copy(w1f[:], w1[:])
                w2f = small.tile([128, 96], f32, tag=f"w2f{name}")
                nc.vector.tensor_tensor(w2f[:], Wf[:], w1f[:],
                                        op=mybir.AluOpType.subtract)
                w2 = small.tile([128, 96], bf16, tag=f"w2{name}")
                nc.vector.tensor_copy(w2[:], w2f[:])
                W1[name], W2[name] = w1, w2
            tc.cur_priority += 1000

            # ---------------- one-hot + gather + scan ----------------
            oh_lo = big.tile([128, SLOT], bf16)
            oh_hi = big.tile([128, SLOT], bf16)

            # single unified output tile: rows [0:64) hv+hi, rows [64:96)
            # the lik prefix sums (same base partition as the PSUM lik rows),
            # rows [96:128) never written (host ignores them)
            outt = big.tile([128, SLOT], f32)
            gath = outt[0:64]
            lcum = outt[64:96]
            off = 0
            ci = 0
            while off < SLOT:
                n = min(NMM, SLOT - off)
                sl = slice(off, off + n)
                nc.vector.tensor_scalar(oh_lo[:, sl], xb[:, sl], ilo[:, 0:1],
                                        None, op0=mybir.AluOpType.is_equal)
                nc.vector.tensor_scalar(oh_hi[:, sl], xb[:, sl], ihi[:, 0:1],
                                        None, op0=mybir.AluOpType.is_equal)
                ps = psum.tile([96, n], f32)
                nc.tensor.matmul(ps[:], W1["lo"], oh_lo[:, sl], start=True, stop=False)
                nc.tensor.matmul(ps[:], W1["hi"], oh_hi[:, sl], start=False, stop=False)
                nc.tensor.matmul(ps[:], W2["lo"], oh_lo[:, sl], start=False, stop=False)
                nc.tensor.matmul(ps[:], W2["hi"], oh_hi[:, sl], start=False, stop=True)
                # evacuate hv+hi rows via ACT; lik rows go straight from
                # PSUM through the chained prefix scan on DVE
                nc.scalar.copy(gath[0:64, sl], ps[0:64, :])
                init = 0.0 if off == 0 else lcum[:, off - 1:off]
                nc.vector.tensor_tensor_scan(
                    lcum[:, sl], ps[64:96, :], lcum[:, sl], init,
                    op0=mybir.AluOpType.add, op1=mybir.AluOpType.bypass)
                off += n
                ci += 1

            # staged output DMAs: early pieces overlap the matmul pipeline,
            # alternating between the SP and Pool queues
            h1, h2, h3 = 6 * NMM, 12 * NMM, 16 * NMM
            nc.sync.dma_start(out_ext[:, 0:h1], outt[:, 0:h1])
            nc.gpsimd.dma_start(out_ext[:, h1:h2], outt[:, h1:h2])
            nc.sync.dma_start(out_ext[:, h2:h3], outt[:, h2:h3])
            nc.gpsimd.dma_start(out_ext[:, h3:SLOT], outt[:, h3:SLOT])


def _get_nc():
    if "nc" not in _cache:
        _cache["nc"] = _build_nc()
    return _cache["nc"]


def _host_prep(B, Pi, x):
    Bp = np.ascontiguousarray(B.transpose(0, 2, 1).reshape(C, NGEN * M))
    ilo = (np.arange(128) % 32).astype(np.float32).reshape(128, 1)
    ihi = ilo + 32.0
    iotac = np.tile((np.arange(32, dtype=np.float32) + BIG)[None, :],
                    (C, 16)).astype(np.float32)
    x_i = x.astype(np.int64)
    in_maps = []
    for c in range(NCORES):
        xc = x_i[c * NPC:(c + 1) * NPC]
        xc = np.concatenate([xc, np.zeros(NSLOT * SLOT - NPC, np.int64)])
        slots = xc.reshape(NSLOT, 1, SLOT).astype(ml_dtypes.bfloat16)
        xb = np.broadcast_to(slots, (NSLOT, 32, SLOT)).reshape(128, SLOT)
        in_maps.append({
            "Bp": Bp, "Pip": Pi, "xb": np.ascontiguousarray(xb),
            "ilo": ilo, "ihi": ihi, "iotac": iotac,
        })
    return in_maps


def _host_post(res, batch):
    hv = np.empty((N, NGEN), np.float32)
    hi = np.empty((N, NGEN), np.float32)
    lcum = np.empty((NCORES, NGEN, NSLOT, SLOT), np.float64)
    for c in range(NCORES):
        out = res[c]["out"]              # [128, SLOT], row = 4j + s (+ block)
        hv_c = out[0:32].reshape(NGEN, NSLOT, SLOT)
        hi_c = out[32:64].reshape(NGEN, NSLOT, SLOT)
        # node local = SLOT*s + f
        hv[c * NPC:(c + 1) * NPC] = hv_c.transpose(1, 2, 0).reshape(-1, NGEN)[:NPC]
        hi[c * NPC:(c + 1) * NPC] = hi_c.transpose(1, 2, 0).reshape(-1, NGEN)[:NPC]
        lcum[c] = out[64:96].reshape(NGEN, NSLOT, SLOT)

    # chunk (= slot) totals at last real node of each slot
    last_f = [SLOT - 1, SLOT - 1, SLOT - 1, NPC - 3 * SLOT - 1]
    tot = np.stack([lcum[:, :, s, last_f[s]] for s in range(NSLOT)],
                   axis=2)                      # (cores, NGEN, NSLOT)
    flat = tot.transpose(0, 2, 1).reshape(NCORES * NSLOT, NGEN)
    prefix = np.concatenate([np.zeros((1, NGEN)), np.cumsum(flat, 0)[:-1]],
                            0).reshape(NCORES, NSLOT, NGEN)

    bounds = np.searchsorted(batch, np.arange(G), side="right")

    def prefix_at(p):
        out = np.zeros((len(p), NGEN), np.float64)
        nz = p > 0
        i = p[nz] - 1
        cc = i // NPC
        rr = i % NPC
        ss = rr // SLOT
        ff = rr % SLOT
        # advanced indices separated by the ":" slice -> shape (n, NGEN)
        out[nz] = prefix[cc, ss] + lcum[cc, :, ss, ff]
        return out

    Pb = prefix_at(bounds)
    Pb_prev = np.concatenate([np.zeros((1, NGEN)), Pb[:-1]], axis=0)
    likelihood = (Pb - Pb_prev).astype(np.float32)
    h_vals = hv.reshape(N, 1, NGEN)
    h_idx = np.rint(hi).astype(np.int32)
    return likelihood, h_vals, h_idx


def kernel(B, Pi, x, batch, num_graphs):
    B = np.asarray(B, dtype=np.float32)
    Pi = np.asarray(Pi, dtype=np.float32)
    x = np.asarray(x)
    batch = np.asarray(batch)
    assert B.shape == (C, M, NGEN) and Pi.shape == (C, NGEN)
    assert x.shape == (N,) and batch.shape == (N,) and int(num_graphs) == G

    nc = _get_nc()
    in_maps = _host_prep(B, Pi, x)
    res = run_bass_kernel_spmd(nc, in_maps, list(range(NCORES))).results
    return _host_post(res, batch)


# revision 20
# speedup vs baseline: 12595.0561x; 12595.0561x over previous
"""CGMM layer (segment_reduce) Trainium2 kernel.

Math: every per-node quantity depends on the node only through its discrete
label x_n (64 values), so the kernel reduces to:
  1. a tiny 64x24 table build from B/Pi on device (softmaxes, posterior,
     max/argmax over C via DVE 32x32 block transposes, per-label likelihood),
  2. a table gather for all N nodes done on the tensor engine: one-hot(x)
     built by DVE is_equal against a per-partition label column (x is fed
     pre-replicated across 32 partitions, 4 node-slots per column), then
     block-diagonal matmuls; fp32 table precision is kept with a bf16 hi/lo
     weight split accumulated in PSUM,
  3. an unsegmented prefix sum (DVE tensor_tensor_scan) of the per-node
     likelihood; the host samples it at graph boundaries (known from the
     sorted `batch`) and differences to get per-graph segment sums.

Sharding: data-parallel over nodes, 37500 nodes per core on 8 cores; B/Pi
replicated; outputs disjoint per core except graph sums, stitched on host.

Output column layout of the gather matmul (96 rows): col = 4*j + s for
output j (0..7 h_vals, 8..15 h_idx, 16..23 lik) and node-slot s (0..3),
so h_vals rows = [0:32), h_idx = [32:64), lik = [64:96) - contiguous blocks.
"""

import numpy as np
import ml_dtypes
from contextlib import ExitStack

import concourse.bass as bass
import concourse.tile as tile
import concourse.mybir as mybir
from concourse import bacc
from concourse.bass_utils import run_bass_kernel_spmd

N = 300000
C = 32
M = 64
NGEN = 8
G = 8192
NCORES = 8
NPC = N // NCORES          # 37500 nodes per core
SLOT = 9376                # nodes per slot; 4 slots/core, last 4 nodes pad
NSLOT = 4
BIG = 10000.0
NMM = 512                  # matmul free-dim chunk

_cache = {}


def _build_nc(reps: int = 1):
    nc = bacc.Bacc("TRN2", target_bir_lowering=False, debug=False,
                   num_devices=NCORES)
    f32, bf16 = mybir.dt.float32, mybir.dt.bfloat16

    Bp = nc.declare_dram_parameter("Bp", [C, M * NGEN], f32, isOutput=False)
    Pip = nc.declare_dram_parameter("Pip", [C, NGEN], f32, isOutput=False)
    xb_in = nc.declare_dram_parameter("xb", [128, SLOT], bf16, isOutput=False)
    ilo_in = nc.declare_dram_parameter("ilo", [128, 1], f32, isOutput=False)
    ihi_in = nc.declare_dram_parameter("ihi", [128, 1], f32, isOutput=False)
    iotac_in = nc.declare_dram_parameter("iotac", [C, M * NGEN], f32, isOutput=False)

    out_ext = nc.declare_dram_parameter("out", [128, SLOT], f32, isOutput=True)

    with tile.TileContext(nc, num_cores=NCORES) as tc:
        with ExitStack() as ctx:
            small = ctx.enter_context(tc.tile_pool(name="small", bufs=1))
            big = ctx.enter_context(tc.tile_pool(name="big", bufs=1))
            psum = ctx.enter_context(tc.tile_pool(name="ps", bufs=2, space="PSUM"))
            dpool = ctx.enter_context(tc.tile_pool(name="dscratch", bufs=1, space="DRAM"))
            dW = dpool.tile([M, 24], f32)

            import contextlib
            loop_ctx = tc.For_i(0, reps, 1) if reps > 1 else contextlib.nullcontext()
            with loop_ctx:
                _emit_body(nc, tc, locals())

    nc.finalize()
    return nc


def _emit_body(nc, tc, env):
    f32, bf16 = mybir.dt.float32, mybir.dt.bfloat16
    small, big, psum = env["small"], env["big"], env["psum"]
    dW = env["dW"]
    Bp, Pip, xb_in = env["Bp"], env["Pip"], env["xb_in"]
    ilo_in, ihi_in, iotac_in = env["ilo_in"], env["ihi_in"], env["iotac_in"]
    out_ext = env["out_ext"]
    if True:
        if True:
            # ---------------- inputs ----------------
            xb = big.tile([128, SLOT], bf16)
            nc.sync.dma_start(xb[:], xb_in[:])
            ilo = small.tile([128, 1], f32)
            nc.gpsimd.dma_start(ilo[:], ilo_in[:])
            ihi = small.tile([128, 1], f32)
            nc.gpsimd.dma_start(ihi[:], ihi_in[:])
            iotac = small.tile([C, 512], f32)
            nc.gpsimd.dma_start(iotac[:], iotac_in[:])
            Bt = small.tile([C, 512], f32)
            nc.scalar.dma_start(Bt[:], Bp[:])
            Pit = small.tile([C, NGEN], f32)
            nc.scalar.dma_start(Pit[:], Pip[:])

            # ---------------- table build (tiny) ----------------
            warm = small.tile([1, 8], f32)
            nc.vector.memset(warm[:], 0.0)
            nc.scalar.activation(warm[:], warm[:],
                                 mybir.ActivationFunctionType.Exp)
            tc.cur_priority -= 1000
            eB = small.tile([C, 512], f32)
            nc.scalar.activation(eB[:], Bt[:], mybir.ActivationFunctionType.Exp)
            sB = small.tile([C, NGEN], f32)
            nc.vector.tensor_reduce(
                sB[:], eB[:].rearrange("p (g m) -> p g m", m=M),
                axis=mybir.AxisListType.X, op=mybir.AluOpType.add)
            rB = small.tile([C, NGEN], f32)
            nc.vector.reciprocal(rB[:], sB[:])

            ePi = small.tile([C, NGEN], f32)
            nc.scalar.activation(ePi[:], Pit[:], mybir.ActivationFunctionType.Exp)
            piP = small.tile([C, 32], f32)
            nc.vector.memset(piP[:], 0.0)
            nc.vector.tensor_copy(piP[:, 0:NGEN], ePi[:])
            piT = small.tile([C, 32], f32)
            nc.vector.transpose(piT[:], piP[:])          # piT[g, c]
            sPi = small.tile([C, 1], f32)
            nc.vector.tensor_reduce(sPi[:], piT[:], axis=mybir.AxisListType.X,
                                    op=mybir.AluOpType.add)
            rPi = small.tile([C, 1], f32)
            nc.vector.reciprocal(rPi[:], sPi[:])
            smPiT = small.tile([C, 32], f32)
            nc.vector.tensor_scalar(smPiT[:], piT[:], rPi[:, 0:1], None,
                                    op0=mybir.AluOpType.mult)
            smPi32 = small.tile([C, 32], f32)
            nc.vector.transpose(smPi32[:], smPiT[:])     # smPi32[c, g]

            # num = eB * (rB * smPi) broadcast over m
            fPi = small.tile([C, NGEN], f32)
            nc.vector.tensor_tensor(fPi[:], rB[:], smPi32[:, 0:NGEN],
                                    op=mybir.AluOpType.mult)
            num = small.tile([C, 512], f32)
            nc.vector.tensor_tensor(
                num[:].rearrange("p (g m) -> p g m", m=M),
                eB[:].rearrange("p (g m) -> p g m", m=M),
                fPi[:].unsqueeze(2).broadcast_to((C, NGEN, M)),
                op=mybir.AluOpType.mult)

            # transpose to (mg-within-block, c) for the C-axis reductions
            numT = small.tile([C, 512], f32)
            nc.vector.transpose(numT[:], num[:])
            den = small.tile([C, 16], f32)
            nc.vector.tensor_reduce(
                den[:], numT[:].rearrange("p (k q) -> p k q", q=32),
                axis=mybir.AxisListType.X, op=mybir.AluOpType.add)
            rden = small.tile([C, 16], f32)
            nc.vector.reciprocal(rden[:], den[:])
            postT = small.tile([C, 512], f32)
            nc.vector.tensor_tensor(
                postT[:].rearrange("p (k q) -> p k q", q=32),
                numT[:].rearrange("p (k q) -> p k q", q=32),
                rden[:].unsqueeze(2).broadcast_to((C, 16, 32)),
                op=mybir.AluOpType.mult)
            lognT = small.tile([C, 512], f32)
            nc.scalar.activation(lognT[:], numT[:], mybir.ActivationFunctionType.Ln)
            plT = small.tile([C, 512], f32)
            nc.vector.tensor_tensor(plT[:], postT[:], lognT[:],
                                    op=mybir.AluOpType.mult)
            likmg = small.tile([C, 16], f32)
            nc.vector.tensor_reduce(
                likmg[:], plT[:].rearrange("p (k q) -> p k q", q=32),
                axis=mybir.AxisListType.X, op=mybir.AluOpType.add)
            hvmg = small.tile([C, 16], f32)
            nc.vector.tensor_reduce(
                hvmg[:], postT[:].rearrange("p (k q) -> p k q", q=32),
                axis=mybir.AxisListType.X, op=mybir.AluOpType.max)
            mask = small.tile([C, 512], f32)
            nc.vector.tensor_tensor(
                mask[:].rearrange("p (k q) -> p k q", q=32),
                postT[:].rearrange("p (k q) -> p k q", q=32),
                hvmg[:].unsqueeze(2).broadcast_to((C, 16, 32)),
                op=mybir.AluOpType.is_equal)
            cand2 = small.tile([C, 512], f32)
            nc.vector.scalar_tensor_tensor(cand2[:], mask[:], -BIG, iotac[:],
                                           op0=mybir.AluOpType.mult,
                                           op1=mybir.AluOpType.add)
            himg = small.tile([C, 16], f32)
            nc.vector.tensor_reduce(
                himg[:], cand2[:].rearrange("p (k q) -> p k q", q=32),
                axis=mybir.AxisListType.X, op=mybir.AluOpType.min)

            # dump tables into dW[m, 8t+g]: tile element (p=8a+b, k) is the
            # value for mg = 32k + p, i.e. m = 4k + a, g = b
            dWv = dW[:, :].rearrange("(mhi p) (t g) -> t mhi p g", mhi=2, g=8)
            dump_engs = (nc.sync, nc.scalar, nc.gpsimd)
            for t, tmg in enumerate((hvmg, himg, likmg)):
                tv = tmg[:, :].rearrange("p (g mhi) -> mhi p g", mhi=2)
                for mhi in range(2):
                    dump_engs[t].dma_start(dWv[t, mhi], tv[mhi])

            # fp32 block-diagonal weights, then bf16 hi/lo split.
            # W[32s + m, 4j + s] = T[m, j]
            Wlo = small.tile([128, 96], f32)
            nc.vector.memset(Wlo[:], 0.0)
            Whi = small.tile([128, 96], f32)
            nc.vector.memset(Whi[:], 0.0)
            for s in range(NSLOT):
                dst = slice(32 * s, 32 * s + 32)
                wv_lo = Wlo[dst, :].rearrange("m (j four) -> m four j", four=4)
                wv_hi = Whi[dst, :].rearrange("m (j four) -> m four j", four=4)
                nc.sync.dma_start(wv_lo[:, s], dW[0:32, :])
                nc.scalar.dma_start(wv_hi[:, s], dW[32:64, :])
            W1 = {}
            W2 = {}
            for name, Wf in (("lo", Wlo), ("hi", Whi)):
                w1 = small.tile([128, 96], bf16, tag=f"w1{name}")
                nc.vector.tensor_copy(w1[:], Wf[:])
                w1f = small.tile([128, 96], f32, tag=f"w1f{name}")
                nc.vector.tensor_copy(w1f[:], w1[:])
                w2f = small.tile([128, 96], f32, tag=f"w2f{name}")
                nc.vector.tensor_tensor(w2f[:], Wf[:], w1f[:],
                                        op=mybir.AluOpType.subtract)
                w2 = small.tile([128, 96], bf16, tag=f"w2{name}")
                nc.vector.tensor_copy(w2[:], w2f[:])
                W1[name], W2[name] = w1, w2
            tc.cur_priority += 1000

            # ---------------- one-hot + gather + scan ----------------
            oh_lo = big.tile([128, SLOT], bf16)
            oh_hi = big.tile([128, SLOT], bf16)

            # single unified output tile: rows [0:64) hv+hi, rows [64:96)
            # the lik prefix sums (same base partition as the PSUM lik rows),
            # rows [96:128) never written (host ignores them)
            outt = big.tile([128, SLOT], f32)
            gath = outt[0:64]
            lcum = outt[64:96]
            # super-chunks spanning 4 PSUM banks: one EQ pair, 16 matmuls,
            # one ACT evacuation and one chained DVE scan per super-chunk
            SUPER = 4 * NMM
            off = 0
            while off < SLOT:
                n = min(SUPER, SLOT - off)
                sl = slice(off, off + n)
                nc.vector.tensor_scalar(oh_lo[:, sl], xb[:, sl], ilo[:, 0:1],
                                        None, op0=mybir.AluOpType.is_equal)
                nc.vector.tensor_scalar(oh_hi[:, sl], xb[:, sl], ihi[:, 0:1],
                                        None, op0=mybir.AluOpType.is_equal)
                ps = psum.tile([96, n], f32)
                o2 = 0
                while o2 < n:
                    m = min(NMM, n - o2)
                    s2 = slice(off + o2, off + o2 + m)
                    pv = ps[:, o2:o2 + m]
                    nc.tensor.matmul(pv, W1["lo"], oh_lo[:, s2], start=True, stop=False)
                    nc.tensor.matmul(pv, W1["hi"], oh_hi[:, s2], start=False, stop=False)
                    nc.tensor.matmul(pv, W2["lo"], oh_lo[:, s2], start=False, stop=False)
                    nc.tensor.matmul(pv, W2["hi"], oh_hi[:, s2], start=False, stop=True)
                    o2 += m
                nc.scalar.copy(gath[0:64, sl], ps[0:64, :])
                init = 0.0 if off == 0 else lcum[:, off - 1:off]
                nc.vector.tensor_tensor_scan(
                    lcum[:, sl], ps[64:96, :], lcum[:, sl], init,
                    op0=mybir.AluOpType.add, op1=mybir.AluOpType.bypass)
                off += n

            # staged output DMAs aligned to super-chunk boundaries so each
            # piece fires as soon as its chunk's evac+scan complete,
            # alternating between the SP and Pool queues
            edges = list(range(0, SLOT, SUPER)) + [SLOT]
            for i in range(len(edges) - 1):
                eng = nc.sync if i % 2 == 0 else nc.gpsimd
                eng.dma_start(out_ext[:, edges[i]:edges[i + 1]],
                              outt[:, edges[i]:edges[i + 1]])


def _get_nc():
    if "nc" not in _cache:
        _cache["nc"] = _build_nc()
    return _cache["nc"]


def _host_prep(B, Pi, x):
    Bp = np.ascontiguousarray(B.transpose(0, 2, 1).reshape(C, NGEN * M))
    ilo = (np.arange(128) % 32).astype(np.float32).reshape(128, 1)
    ihi = ilo + 32.0
    iotac = np.tile((np.arange(32, dtype=np.float32) + BIG)[None, :],
                    (C, 16)).astype(np.float32)
    x_i = x.astype(np.int64)
    in_maps = []
    for c in range(NCORES):
        xc = x_i[c * NPC:(c + 1) * NPC]
        xc = np.concatenate([xc, np.zeros(NSLOT * SLOT - NPC, np.int64)])
        slots = xc.reshape(NSLOT, 1, SLOT).astype(ml_dtypes.bfloat16)
        xb = np.broadcast_to(slots, (NSLOT, 32, SLOT)).reshape(128, SLOT)
        in_maps.append({
            "Bp": Bp, "Pip": Pi, "xb": np.ascontiguousarray(xb),
            "ilo": ilo, "ihi": ihi, "iotac": iotac,
        })
    return in_maps


def _host_post(res, batch):
    hv = np.empty((N, NGEN), np.float32)
    hi = np.empty((N, NGEN), np.float32)
    lcum = np.empty((NCORES, NGEN, NSLOT, SLOT), np.float64)
    for c in range(NCORES):
        out = res[c]["out"]              # [128, SLOT], row = 4j + s (+ block)
        hv_c = out[0:32].reshape(NGEN, NSLOT, SLOT)
        hi_c = out[32:64].reshape(NGEN, NSLOT, SLOT)
        # node local = SLOT*s + f
        hv[c * NPC:(c + 1) * NPC] = hv_c.transpose(1, 2, 0).reshape(-1, NGEN)[:NPC]
        hi[c * NPC:(c + 1) * NPC] = hi_c.transpose(1, 2, 0).reshape(-1, NGEN)[:NPC]
        lcum[c] = out[64:96].reshape(NGEN, NSLOT, SLOT)

    # chunk (= slot) totals at last real node of each slot
    last_f = [SLOT - 1, SLOT - 1, SLOT - 1, NPC - 3 * SLOT - 1]
    tot = np.stack([lcum[:, :, s, last_f[s]] for s in range(NSLOT)],
                   axis=2)                      # (cores, NGEN, NSLOT)
    flat = tot.transpose(0, 2, 1).reshape(NCORES * NSLOT, NGEN)
    prefix = np.concatenate([np.zeros((1, NGEN)), np.cumsum(flat, 0)[:-1]],
                            0).reshape(NCORES, NSLOT, NGEN)

    bounds = np.searchsorted(batch, np.arange(G), side="right")

    def prefix_at(p):
        out = np.zeros((len(p), NGEN), np.float64)
        nz = p > 0
        i = p[nz] - 1
        cc = i // NPC
        rr = i % NPC
        ss = rr // SLOT
        ff = rr % SLOT
        # advanced indices separated by the ":" slice -> shape (n, NGEN)
        out[nz] = prefix[cc, ss] + lcum[cc, :, ss, ff]
        return out

    Pb = prefix_at(bounds)
    Pb_prev = np.concatenate([np.zeros((1, NGEN)), Pb[:-1]], axis=0)
    likelihood = (Pb - Pb_prev).astype(np.float32)
    h_vals = hv.reshape(N, 1, NGEN)
    h_idx = np.rint(hi).astype(np.int32)
    return likelihood, h_vals, h_idx


def kernel(B, Pi, x, batch, num_graphs):
    B = np.asarray(B, dtype=np.float32)
    Pi = np.asarray(Pi, dtype=np.float32)
    x = np.asarray(x)
    batch = np.asarray(batch)
    assert B.shape == (C, M, NGEN) and Pi.shape == (C, NGEN)
    assert x.shape == (N,) and batch.shape == (N,) and int(num_graphs) == G

    nc = _get_nc()
    in_maps = _host_prep(B, Pi, x)
    res = run_bass_kernel_spmd(nc, in_maps, list(range(NCORES))).results
    return _host_post(res, batch)
